# revision 1
# baseline (speedup 1.0000x reference)
import numpy as np
import jax
import jax.numpy as jnp

# nn_GatedFusionBlockCustom: B=8, S=2048, H=256, NH=8 heads.
# Data-parallel over batch: one batch element per NeuronCore (8 cores),
# weights replicated. The global gating mean-pool is per-batch-element,
# so the whole block runs with zero cross-core communication.
B, S, H, NH = 8, 2048, 256, 8
DH = H // NH

WEIGHT_KEYS = [
    'g_mha_w1', 'g_mha_b1', 'g_mha_w2', 'g_mha_b2',
    'g_ffn_w1', 'g_ffn_b1', 'g_ffn_w2', 'g_ffn_b2',
    'aproj_w', 'aproj_b', 'outproj_w', 'outproj_b',
    'ffn1_w1', 'ffn1_b1', 'ffn1_w2', 'ffn1_b2',
    'ffn2_w1', 'ffn2_b1', 'ffn2_w2', 'ffn2_b2',
    'attn_in_w', 'attn_in_b', 'attn_out_w', 'attn_out_b',
    'n1_g', 'n1_b', 'n2_g', 'n2_b', 'n3_g', 'n3_b', 'n4_g', 'n4_b',
]


def _lin(x, w, b):
    return x @ w.T + b


def _ln(x, g, b, eps=1e-5):
    mu = x.mean(-1, keepdims=True)
    var = ((x - mu) ** 2).mean(-1, keepdims=True)
    return (x - mu) / jnp.sqrt(var + eps) * g + b


def _block(video_feat, audio_feat, p):
    # video_feat/audio_feat: [S, H] — one batch element on this core.
    joint = jnp.concatenate([video_feat.mean(0), audio_feat.mean(0)])  # [2H]
    gate_mha = jnp.tanh(_lin(jax.nn.relu(_lin(joint, p['g_mha_w1'], p['g_mha_b1'])),
                             p['g_mha_w2'], p['g_mha_b2']))  # [1]
    gate_ffn = jnp.tanh(_lin(jax.nn.relu(_lin(joint, p['g_ffn_w1'], p['g_ffn_b1'])),
                             p['g_ffn_w2'], p['g_ffn_b2']))  # [1]
    gm = gate_mha[0]
    gf = gate_ffn[0]

    norm_audio = _ln(audio_feat, p['n1_g'], p['n1_b'])
    attn_output = _lin(_lin(norm_audio, p['aproj_w'], p['aproj_b']),
                       p['outproj_w'], p['outproj_b'])
    z = gm * attn_output + video_feat

    h1 = _lin(jax.nn.relu(_lin(_ln(z, p['n2_g'], p['n2_b']), p['ffn1_w1'], p['ffn1_b1'])),
              p['ffn1_w2'], p['ffn1_b2'])
    z_bar = gf * h1 + z

    x3 = _ln(z_bar, p['n3_g'], p['n3_b'])
    qkv = _lin(x3, p['attn_in_w'], p['attn_in_b'])  # [S, 3H]
    q, k, v = jnp.split(qkv, 3, axis=-1)
    q = q.reshape(S, NH, DH)
    k = k.reshape(S, NH, DH)
    v = v.reshape(S, NH, DH)
    scores = jnp.einsum('qhd,khd->hqk', q, k) * (DH ** -0.5)
    attn = jax.nn.softmax(scores, axis=-1)
    ctx = jnp.einsum('hqk,khd->qhd', attn, v).reshape(S, H)
    refined_z = _lin(ctx, p['attn_out_w'], p['attn_out_b']) + z_bar

    final = _lin(jax.nn.relu(_lin(_ln(refined_z, p['n4_g'], p['n4_b']),
                                  p['ffn2_w1'], p['ffn2_b1'])),
                 p['ffn2_w2'], p['ffn2_b2']) + refined_z

    gate_mha_full = jnp.full((S, H), gm, jnp.float32)
    gate_ffn_full = jnp.full((S, H), gf, jnp.float32)
    return final, gate_mha_full, gate_ffn_full


_pblock = jax.pmap(_block, in_axes=(0, 0, None))


def kernel(**inputs):
    video = jnp.asarray(np.asarray(inputs['video_feat'], np.float32))
    audio = jnp.asarray(np.asarray(inputs['audio_feat'], np.float32))
    params = {k: jnp.asarray(np.asarray(inputs[k], np.float32)) for k in WEIGHT_KEYS}
    final, gmf, gff = _pblock(video, audio, params)
    return (np.asarray(final, np.float32),
            np.asarray(gmf, np.float32),
            np.asarray(gff, np.float32))
